# revision 34
# baseline (speedup 1.0000x reference)
"""Trainium2 Bass kernel for nn_MinervaEnhancedLoss (8-core data-parallel).

Distribution: pure data parallel over batch; 64 samples per core.
Device layout: partition p = (h, s) with h in {0,1} pixel-halves, s in
0..63 -> 128 partitions; J = 2048 pixels per partition row.

Host pre-transposes pred to [C, P, J] per core so each channel is one
fully-contiguous 1 MB DMA (8 KB per-partition rows -> max HBM streaming
efficiency), and targets to [P, J] float16.

Per core, pipelined per channel c = 0..9 (channels 0, 1, 9 processed in
J-halves to shorten the pipeline ramp and tail):
  - DMA channel c (1 MB, HWDGE, sync queue)
  - exp -> fp16 on the scalar engine
  - S += E_c and Et += (t==c)*E_c via identity-matmul PSUM accumulation
    (one-hot masks built on the DVE, interleaved so the PE-critical
    prod never queues behind them)
  - bit-tagged uint16 running max for argmax (low 4 mantissa bits of the
    fp16 exp value replaced by the channel id; host applies the & 0xF)
Then the focal chain per J-half as soon as its PSUM half completes:
ce = ln S - ln Et, pt = exp(-ce), (1-pt)^2 on the scalar engine, and a
fused multiply + row-sum accumulate (scalar_tensor_tensor) into stats.

Host side: intersection / copy-match / exact counts from the returned
argmax map, unique-color weights, diversity codes, creativity, and the
final loss formulas (all cheap O(B*H*W) numpy).
"""

import sys

sys.path.insert(0, "/opt/trn_rl_repo")

import numpy as np

import concourse.bass as bass
import concourse.mybir as mybir
from concourse import tile
from concourse.bass_utils import run_bass_kernel_spmd

AF = mybir.ActivationFunctionType
ALU = mybir.AluOpType
DT = mybir.dt

NCORES = 8
B, C, H, W = 512, 10, 64, 64
BS = B // NCORES          # 64 samples per core
PIX = H * W               # 4096 pixels per sample
HALF = 2                  # pixel halves per sample -> partition = (h, s)
J = PIX // HALF           # 2048 pixels per partition
P = BS * HALF             # 128 partitions
MMW = 512                 # matmul moving-operand width (1 PSUM bank)
JH = J // 2               # focal-chain half width

NUM_CLASSES = 10
LABEL_SMOOTHING = 0.1
GAMMA = 2.0
TRANSFORM_PENALTY = 0.2
EXACT_MATCH_BONUS = 5.0
CREATIVITY_WEIGHT = 0.15

_compiled = None


def _legalize_ctrl_waits(nc, max_waits=1):
    """Split >max_waits sem-waits on ctrl instructions onto preceding NoOps.

    This walrus build rejects Drain/NoOp instructions with more than a couple
    of sync-wait commands; Tile's tail drain can carry three or more.
    """
    for fn in nc.m.functions:
        for blk in fn.blocks:
            insts = blk.instructions
            new = []
            changed = False
            for inst in insts:
                si = inst.sync_info
                if (
                    si is not None
                    and si.on_wait is not None
                    and len(si.on_wait) > max_waits
                ):
                    waits = list(si.on_wait)
                    extra, keep = waits[:-max_waits], waits[-max_waits:]
                    for j, w in enumerate(extra):
                        new.append(
                            mybir.InstNoOp(
                                name=f"{inst.name}-waitsplit{j}",
                                engine=inst.engine,
                                ins=[],
                                outs=[],
                                sync_info=mybir.SyncInfo(
                                    on_wait=[w], on_update=[]
                                ),
                            )
                        )
                    inst.sync_info = mybir.SyncInfo(
                        on_wait=keep, on_update=list(si.on_update or [])
                    )
                    changed = True
                new.append(inst)
            if changed:
                blk.instructions[:] = new


def _build_program():
    """Build the single-core SPMD Bass program (same NEFF on all 8 cores)."""
    nc = bass.Bass()

    pred = nc.declare_dram_parameter(
        "pred", [C, P, J], DT.float32, isOutput=False
    )
    targ = nc.declare_dram_parameter("targ", [P, J], DT.float16, isOutput=False)
    ident = nc.declare_dram_parameter(
        "ident", [128, 128], DT.float16, isOutput=False
    )
    am_out = nc.declare_dram_parameter("am", [P, J], DT.uint16, isOutput=True)
    stats_out = nc.declare_dram_parameter(
        "stats", [P, 2], DT.float32, isOutput=True
    )

    with tile.TileContext(nc) as tc:
        with (
            tc.tile_pool(name="xin", bufs=6) as xin_pool,
            tc.tile_pool(name="exp", bufs=4) as exp_pool,
            tc.tile_pool(name="prod", bufs=2) as prod_pool,
            tc.tile_pool(name="tag", bufs=5) as tag_pool,
            tc.tile_pool(name="mrun", bufs=2) as mrun_pool,
            tc.tile_pool(name="chain", bufs=2) as chain_pool,
            tc.tile_pool(name="persist", bufs=1) as persist_pool,
            tc.tile_pool(name="psum_s", bufs=1, space=bass.MemorySpace.PSUM) as ps_pool,
            tc.tile_pool(name="psum_e", bufs=1, space=bass.MemorySpace.PSUM) as pe_pool,
        ):
            # --- one-time loads -------------------------------------------
            t_f16 = persist_pool.tile([P, J], DT.float16)
            nc.sync.dma_start(t_f16[:], targ[:])
            ident_t = persist_pool.tile([128, 128], DT.float16)
            nc.sync.dma_start(ident_t[:], ident[:])

            am_u16 = persist_pool.tile([P, J], DT.uint16)
            stats = persist_pool.tile([P, 2], DT.float32)
            negone = persist_pool.tile([P, 1], DT.float32)
            nc.vector.memset(negone[:], -1.0)

            # one-hot target masks: first few upfront (covers the DMA fill
            # window), the rest interleaved one per channel so prod_c never
            # queues behind them on the DVE
            masks = persist_pool.tile([P, C, J], DT.float16)
            NMASK0 = 3
            for c in range(NMASK0):
                nc.vector.tensor_scalar(
                    masks[:, c, :], t_f16[:], float(c), None,
                    op0=ALU.is_equal,
                )

            # pred channel DMAs (1 MB each, fully contiguous in DRAM)
            x_tiles = []
            for c in range(C):
                x_c = xin_pool.tile([P, J], DT.float32, tag="x")
                nc.sync.dma_start(x_c[:], pred[c, :, :])
                x_tiles.append(x_c)

            psum_s = ps_pool.tile([P, J], DT.float32, tag="s")
            psum_et = pe_pool.tile([P, J], DT.float32, tag="et")

            # running argmax max kept as two J-half lanes (lets ramp/tail
            # channels run at half granularity); tag sources per channel are
            # either a full [P, J] tile or two [P, JH] half tiles
            tagsrc = [None] * C
            mprev = [None, None]

            def half_ap(c, q):
                js = slice(q * JH, (q + 1) * JH)
                src = tagsrc[c]
                if isinstance(src, list):
                    return src[q][:]
                return src[:, js]

            def tree_step(c):
                for q in range(2):
                    src = half_ap(c, q)
                    if mprev[q] is None:
                        mprev[q] = src
                    else:
                        mnew = mrun_pool.tile([P, JH], DT.uint16, tag=f"m{q}")
                        nc.vector.tensor_tensor(
                            mnew[:], mprev[q], src, op=ALU.max
                        )
                        mprev[q] = mnew[:]

            def ch_half(c, q, start, stop):
                """Half-channel unit: exp, S/Et matmul blocks, prod, tag."""
                js = slice(q * JH, (q + 1) * JH)
                e_h = exp_pool.tile([P, JH], DT.float16, tag="eh")
                nc.scalar.activation(e_h[:], x_tiles[c][:, js], AF.Exp)
                for b in range(2 * q, 2 * q + 2):
                    bs_ = slice(b * MMW, (b + 1) * MMW)
                    eb = slice((b - 2 * q) * MMW, (b - 2 * q + 1) * MMW)
                    nc.tensor.matmul(
                        psum_s[:, bs_], ident_t[:], e_h[:, eb],
                        start=start, stop=stop,
                    )
                prod = prod_pool.tile([P, JH], DT.float16, tag="ph")
                nc.vector.tensor_tensor(
                    prod[:], masks[:, c, js], e_h[:], op=ALU.mult
                )
                for b in range(2 * q, 2 * q + 2):
                    bs_ = slice(b * MMW, (b + 1) * MMW)
                    eb = slice((b - 2 * q) * MMW, (b - 2 * q + 1) * MMW)
                    nc.tensor.matmul(
                        psum_et[:, bs_], ident_t[:], prod[:, eb],
                        start=start, stop=stop,
                    )
                yu_h = tag_pool.tile([P, JH], DT.uint16, tag="yh")
                nc.vector.tensor_scalar(
                    yu_h[:], e_h[:].bitcast(DT.uint16), 0xFFF0, c,
                    op0=ALU.bitwise_and, op1=ALU.bitwise_or,
                )
                if tagsrc[c] is None:
                    tagsrc[c] = [None, None]
                tagsrc[c][q] = yu_h

            # --- ramp: channels 0 and 1 in J-halves -----------------------
            for c in range(2):
                for q in range(2):
                    ch_half(c, q, start=(c == 0), stop=False)
                if c + NMASK0 < C:
                    nc.vector.tensor_scalar(
                        masks[:, c + NMASK0, :], t_f16[:],
                        float(c + NMASK0), None, op0=ALU.is_equal,
                    )
                if c > 0:
                    tree_step(c - 1)

            # --- steady channels 2..8 at full width -----------------------
            for c in range(2, C - 1):
                e_c = exp_pool.tile([P, J], DT.float16, tag="e")
                nc.scalar.activation(e_c[:], x_tiles[c][:], AF.Exp)
                for b in range(J // MMW):
                    bs_ = slice(b * MMW, (b + 1) * MMW)
                    nc.tensor.matmul(
                        psum_s[:, bs_], ident_t[:], e_c[:, bs_],
                        start=False, stop=False,
                    )
                prod = prod_pool.tile([P, J], DT.float16, tag="p")
                nc.vector.tensor_tensor(
                    prod[:], masks[:, c, :], e_c[:], op=ALU.mult
                )
                for b in range(J // MMW):
                    bs_ = slice(b * MMW, (b + 1) * MMW)
                    nc.tensor.matmul(
                        psum_et[:, bs_], ident_t[:], prod[:, bs_],
                        start=False, stop=False,
                    )
                if c + NMASK0 < C:
                    nc.vector.tensor_scalar(
                        masks[:, c + NMASK0, :], t_f16[:],
                        float(c + NMASK0), None, op0=ALU.is_equal,
                    )
                yu = tag_pool.tile([P, J], DT.uint16, tag="y")
                nc.vector.tensor_scalar(
                    yu[:], e_c[:].bitcast(DT.uint16), 0xFFF0, c,
                    op0=ALU.bitwise_and, op1=ALU.bitwise_or,
                )
                tagsrc[c] = yu
                tree_step(c - 1)

            # --- tail: channel 9 in J-halves ------------------------------
            c = C - 1

            def am_half(q):
                # final tree step straight into the output tile; the host
                # applies the & 0xF tag mask
                js = slice(q * JH, (q + 1) * JH)
                nc.vector.tensor_tensor(
                    am_u16[:, js], mprev[q], half_ap(c, q), op=ALU.max
                )

            def chain_half(q):
                js = slice(q * JH, (q + 1) * JH)
                ln_s = chain_pool.tile([P, JH], DT.float16, tag="lns")
                nc.scalar.activation(ln_s[:], psum_s[:, js], AF.Ln)
                ln_et = chain_pool.tile([P, JH], DT.float16, tag="lnet")
                nc.scalar.activation(ln_et[:], psum_et[:, js], AF.Ln)
                ce = chain_pool.tile([P, JH], DT.float16, tag="ce")
                nc.vector.tensor_tensor(
                    ce[:], ln_s[:], ln_et[:], op=ALU.subtract
                )
                pt = chain_pool.tile([P, JH], DT.float16, tag="pt")
                nc.scalar.activation(pt[:], ce[:], AF.Exp, scale=-1.0)
                sq = chain_pool.tile([P, JH], DT.float16, tag="sq")
                nc.scalar.activation(sq[:], pt[:], AF.Square, bias=negone[:])
                foc = chain_pool.tile([P, JH], DT.float16, tag="foc")
                nc.vector.scalar_tensor_tensor(
                    foc[:], sq[:], 1.0, ce[:],
                    op0=ALU.mult, op1=ALU.mult,
                    accum_out=stats[:, q : q + 1],
                )

            ch_half(c, 0, start=False, stop=True)
            tree_step(C - 2)
            ch_half(c, 1, start=False, stop=True)
            am_half(0)
            chain_half(0)
            am_half(1)
            nc.sync.dma_start(am_out[:], am_u16[:])
            chain_half(1)
            nc.sync.dma_start(stats_out[:], stats[:])

    _legalize_ctrl_waits(nc)
    return nc


def _get_program():
    global _compiled
    if _compiled is None:
        _compiled = _build_program()
    return _compiled


def _run_device(pred_output, targets, trace=False, **kw):
    nc = _get_program()
    ident_np = np.eye(128, dtype=np.float16)
    # host re-layout: per core pred [s, c, h, j] -> [c, (h, s), j] contiguous
    pred_r = pred_output.reshape(NCORES, BS, C, HALF, J)
    targ_r = targets.astype(np.float16).reshape(NCORES, BS, HALF, J)
    in_maps = []
    for i in range(NCORES):
        in_maps.append(
            {
                "pred": np.ascontiguousarray(
                    pred_r[i].transpose(1, 2, 0, 3).reshape(C, P, J)
                ),
                "targ": np.ascontiguousarray(
                    targ_r[i].transpose(1, 0, 2).reshape(P, J)
                ),
                "ident": ident_np,
            }
        )
    res = run_bass_kernel_spmd(
        nc, in_maps, list(range(NCORES)), trace=trace, **kw
    )
    return res


def _finalize(results, targets, inputs, strategic_reasoning):
    """Host-side reductions from per-core device outputs (all O(B*H*W) cheap)."""
    pred_idx = np.empty((B, PIX), dtype=np.int64)
    focal_s = np.empty(B, dtype=np.float64)
    for i in range(NCORES):
        out = results[i]
        am = (out["am"] & 0xF).reshape(HALF, BS, J).transpose(1, 0, 2).reshape(BS, PIX)
        pred_idx[i * BS : (i + 1) * BS] = am
        st = out["stats"].astype(np.float64).reshape(HALF, BS, 2)
        focal_s[i * BS : (i + 1) * BS] = st.sum(axis=(0, 2))

    targets = targets.astype(np.int64).reshape(B, PIX)
    inputs = inputs.astype(np.int64).reshape(B, PIX)

    # strategic weights from targets
    present = np.zeros((B, NUM_CLASSES), dtype=bool)
    rows = np.repeat(np.arange(B), PIX)
    present[rows, targets.ravel()] = True
    unique_colors = present.sum(axis=1)
    w_s = np.where(unique_colors > 3, 1.2, 1.0)

    focal_loss = (focal_s * w_s).sum() / (B * PIX)

    # exact-match / IoU stats from the device argmax map
    inter_s = (pred_idx == targets).sum(axis=1).astype(np.float64)
    exact_strict = (inter_s == PIX).astype(np.float64)
    iou = inter_s / PIX
    combined = 0.2 * exact_strict + 0.8 * iou
    exact_count = combined.sum()
    exact_bonus = max(-combined.mean() * EXACT_MATCH_BONUS, -3.0)

    copy_s = (pred_idx == inputs).sum(axis=1)
    copy_all = (copy_s == PIX).astype(np.float64)
    transform_penalty = copy_all.mean() * TRANSFORM_PENALTY

    # creativity (tiny input, host)
    sr = strategic_reasoning.astype(np.float64)
    creativity = (1.0 / (1.0 + np.exp(-sr))).mean() * CREATIVITY_WEIGHT

    # diversity: distinct 2x2 codes per sample
    p = pred_idx.reshape(B, H, W)
    codes = (
        p[:, :-1, :-1] * 1000
        + p[:, :-1, 1:] * 100
        + p[:, 1:, :-1] * 10
        + p[:, 1:, 1:]
    ).reshape(B, -1)
    glob = codes + (np.arange(B)[:, None] * 10000)
    cnt = np.bincount(glob.ravel(), minlength=B * 10000)
    n_unique = (cnt.reshape(B, 10000) > 0).sum(axis=1).astype(np.float64)
    diversity = (n_unique / ((H - 1) * (W - 1))).mean() * 0.02

    grid_size_factor = min(H * W / 900.0, 1.0)
    grid_complexity = combined.mean() * grid_size_factor * 0.05

    total = (
        focal_loss
        + transform_penalty
        + exact_bonus
        - creativity
        - diversity
        - grid_complexity
    )
    if np.isnan(total) or np.isinf(total):
        total = min(focal_loss, 10.0)

    out = (
        total,
        focal_loss,
        transform_penalty,
        exact_bonus,
        exact_count,
        combined.sum(),
        iou.mean(),
        creativity,
        diversity,
        grid_complexity,
    )
    return tuple(np.float32(v) for v in out)


def kernel(pred_output, targets, inputs, strategic_reasoning):
    pred_output = np.asarray(pred_output, dtype=np.float32)
    targets = np.asarray(targets)
    inputs = np.asarray(inputs)
    strategic_reasoning = np.asarray(strategic_reasoning, dtype=np.float32)
    res = _run_device(pred_output, targets)
    return _finalize(res.results, targets, inputs, strategic_reasoning)


def kernel_timed(pred_output, targets, inputs, strategic_reasoning, **kw):
    """Like kernel() but traces and returns (outputs, BassKernelResults)."""
    pred_output = np.asarray(pred_output, dtype=np.float32)
    targets = np.asarray(targets)
    inputs = np.asarray(inputs)
    strategic_reasoning = np.asarray(strategic_reasoning, dtype=np.float32)
    res = _run_device(pred_output, targets, trace=True, **kw)
    outs = _finalize(res.results, targets, inputs, strategic_reasoning)
    return outs, res
